# revision 1
# baseline (speedup 1.0000x reference)
"""Trainium2 Bass kernel for the DialogGCN GAT-style message-passing layer.

Math notes (why this is much cheaper than the reference graph):
  Kp    = concat(K, kfeat) @ Wk + bk                    (B,N,D)
  alpha = Q@wden[:D] + Kp@wden[D:] + bden               (B,N)
  w     = softmax(alpha - (1-adj)*1e30, axis=N)
  out   = sum_n w * ((Kp@Wr0)*sm + (Kp@Wr1)*(1-sm))

* softmax is invariant to per-row constants, so the Q term, bden and the
  bk@wden[D:] constant all cancel:  w = softmax_n(X_n . v) masked, where
  X = concat(K, kfeat) and v = Wk @ wden[D:]  (folded on host).
* the output is linear in the weighted sums:
    out = (sum_n w*sm*X_n | c0) @ [Wk;bk] @ Wr0 + (sum_n w*(1-sm)*X_n | c1) @ [Wk;bk] @ Wr1
  so G0 = [Wk;bk]@Wr0 and G1 = [Wk;bk]@Wr1 are folded on host (769x512 each)
  and the device only needs one streaming pass over X computing
    s_n = X_n . v ; p_n = exp(s_n)*adj_n ; U0 = sum p*sm*[X|1] ; U1 = sum p*(1-sm)*[X|1]
  followed by a tiny projection (U0@G0 + U1@G1) / P.

Sharding: pure data parallel over batch B=32 across 8 cores (4 rows each).
"""

import os
import sys

import numpy as np

for _p in ("/opt/trn_rl_repo", "/root/.axon_site/_ro/trn_rl_repo"):
    if os.path.isdir(_p) and _p not in sys.path:
        sys.path.insert(0, _p)

B, N, D, KD = 32, 2048, 512, 256
F = D + KD  # 768
NCORES = 8
BL = B // NCORES  # 4 batch rows per core
NT = 16  # free-dim token tiles per batch (N = 128 * NT)

_BUILD_CACHE = {}
last_results = None  # BassKernelResults of the most recent run (for test.py)


def _build(stream_f32r: bool):
    """Trace the Bass program (same NEFF runs SPMD on all 8 cores)."""
    import concourse.bass as bass
    import concourse.tile as tile
    from concourse import bacc, mybir
    from concourse.masks import make_identity

    f32 = mybir.dt.float32
    i32 = mybir.dt.int32
    mm_dt = mybir.dt.float32r if stream_f32r else f32

    nc = bacc.Bacc()

    # ---- DRAM I/O ----------------------------------------------------------
    # X inputs carry the streaming-matmul dtype (float32r == float32 bits;
    # only the PE interprets it as tf32) so the BIR verifier sees consistent
    # producer/consumer dtypes on the fp32r path.
    xK_f = nc.dram_tensor("xK_f", [BL, N, D], mm_dt, kind="ExternalInput")
    xk1_f = nc.dram_tensor("xk1_f", [BL, N, KD], mm_dt, kind="ExternalInput")
    xK_b = nc.dram_tensor("xK_b", [BL, N, D], mm_dt, kind="ExternalInput")
    xk1_b = nc.dram_tensor("xk1_b", [BL, N, KD], mm_dt, kind="ExternalInput")
    adj_f = nc.dram_tensor("adj_f", [BL, N], i32, kind="ExternalInput")
    sm_f = nc.dram_tensor("sm_f", [BL, N], i32, kind="ExternalInput")
    adj_b = nc.dram_tensor("adj_b", [BL, N], i32, kind="ExternalInput")
    sm_b = nc.dram_tensor("sm_b", [BL, N], i32, kind="ExternalInput")
    v_f = nc.dram_tensor("v_f", [F], f32, kind="ExternalInput")
    v_b = nc.dram_tensor("v_b", [F], f32, kind="ExternalInput")
    G0_f = nc.dram_tensor("G0_f", [F + 1, D], f32, kind="ExternalInput")
    G1_f = nc.dram_tensor("G1_f", [F + 1, D], f32, kind="ExternalInput")
    G0_b = nc.dram_tensor("G0_b", [F + 1, D], f32, kind="ExternalInput")
    G1_b = nc.dram_tensor("G1_b", [F + 1, D], f32, kind="ExternalInput")
    out_f = nc.dram_tensor("out_f", [BL, D], f32, kind="ExternalOutput")
    out_b = nc.dram_tensor("out_b", [BL, D], f32, kind="ExternalOutput")

    branches = [
        dict(xK=xK_f, xk1=xk1_f, adj=adj_f, sm=sm_f, v=v_f, G0=G0_f, G1=G1_f, out=out_f),
        dict(xK=xK_b, xk1=xk1_b, adj=adj_b, sm=sm_b, v=v_b, G0=G0_b, G1=G1_b, out=out_b),
    ]

    with tile.TileContext(nc) as tc:
        with (
            tc.tile_pool(name="singles", bufs=1) as singles,
            tc.tile_pool(name="xKp", bufs=2) as xKp,
            tc.tile_pool(name="xk1p", bufs=3) as xk1p,
            tc.tile_pool(name="scr", bufs=3) as scr,
            tc.tile_pool(name="small", bufs=4) as small,
            tc.tile_pool(name="uallp", bufs=2) as uallp,
            tc.tile_pool(name="uallTp", bufs=2) as uallTp,
            tc.tile_pool(name="finp", bufs=2) as finp,
            tc.tile_pool(name="psU_K", bufs=2, space="PSUM") as psU_K,
            tc.tile_pool(name="psU_1", bufs=2, space="PSUM") as psU_1,
            tc.tile_pool(name="psTr", bufs=2, space="PSUM") as psTr,
            tc.tile_pool(name="psOut", bufs=1, space="PSUM") as psOut,
        ):
            # ---- one-time setup -------------------------------------------
            ident = singles.tile([128, 128], f32)
            make_identity(nc, ident)
            ones11 = singles.tile([1, 1], f32)
            nc.vector.memset(ones11, 1.0)
            # f32 zeros used to produce f32r-typed zeros/ones (memset can't
            # write f32r, but tensor ops can)
            zf = singles.tile([128, NT, 8], f32)
            nc.vector.memset(zf, 0.0)
            # (128,2) ones in the matmul dtype: moving operand of the
            # softmax-denominator accumulation
            ones2 = singles.tile([128, 2], mm_dt)
            nc.vector.tensor_scalar_add(out=ones2, in0=zf[:, 0, 0:2], scalar1=1.0)

            per_br = []
            for br in branches:
                st = {}
                # score vector broadcast across partitions: (128, 768)
                vb = singles.tile([128, F], f32)
                vap = br["v"][:]
                nc.gpsimd.dma_start(
                    out=vb,
                    in_=bass.AP(tensor=vap.tensor, offset=vap.offset, ap=[[0, 128]] + vap.ap),
                )
                st["vb"] = vb
                # G matrices: (128, 7, 512); chunk 6 row 0 holds row 768
                for gname in ("G0", "G1"):
                    g = br[gname]
                    gs = singles.tile([128, 7, D], f32)
                    nc.gpsimd.dma_start(
                        out=gs[:, 0:6, :],
                        in_=g[0:F, :].rearrange("(k p) n -> p k n", p=128),
                    )
                    nc.gpsimd.dma_start(out=gs[0:1, 6, :], in_=g[F : F + 1, :])
                    st[gname] = gs
                # masks for all BL batches: (128, BL, NT), token = p*NT + n
                adj_i = small.tile([128, BL, NT], i32, tag="mask_i")
                sm_i = small.tile([128, BL, NT], i32, tag="mask_i")
                nc.gpsimd.dma_start(out=adj_i, in_=br["adj"].rearrange("b (p n) -> p b n", n=NT))
                nc.gpsimd.dma_start(out=sm_i, in_=br["sm"].rearrange("b (p n) -> p b n", n=NT))
                adjf = small.tile([128, BL, NT], f32, tag="mask_f")
                smf = small.tile([128, BL, NT], f32, tag="mask_f")
                nc.vector.tensor_copy(adjf, adj_i)
                nc.vector.tensor_copy(smf, sm_i)
                m0 = singles.tile([128, BL, NT], f32, tag=f"m0_{br['out'].name}")
                m1 = singles.tile([128, BL, NT], f32, tag=f"m1_{br['out'].name}")
                nc.vector.tensor_mul(m0, adjf, smf)
                nc.vector.tensor_sub(m1, adjf, m0)
                st["m0"], st["m1"] = m0, m1
                per_br.append(st)

            # ---- streaming + finishing per branch -------------------------
            for bi, br in enumerate(branches):
                st = per_br[bi]
                psK = psU_K.tile([8, D], f32)       # rows 0-3: U0(b), rows 4-7: U1(b)
                # cols 0:KD = U_k1, col KD = ones column (P0/P1), col KD+1 = pad
                # (fp32r matmul needs even moving-free-size / 8B alignment)
                ps1 = psU_1.tile([8, KD + 2], f32)

                for b in range(BL):
                    # contiguous-destination tiles keep SWDGE descriptor
                    # generation cheap (strided dst was costing ~13us/unit on Q7)
                    xK = xKp.tile([128, NT, D], mm_dt, tag="xK")
                    nc.gpsimd.dma_start(
                        out=xK, in_=br["xK"][b].rearrange("(p n) d -> p n d", n=NT)
                    )
                    xk1 = xk1p.tile([128, NT, KD], mm_dt, tag="xk1")
                    nc.gpsimd.dma_start(
                        out=xk1, in_=br["xk1"][b].rearrange("(p n) d -> p n d", n=NT)
                    )
                    xK_f32 = xK[:, :, :].bitcast(f32)
                    xk1_f32 = xk1[:, :, :].bitcast(f32)

                    sA = small.tile([128, NT], f32, tag="sA")
                    sB = small.tile([128, NT], f32, tag="sB")
                    prodK = scr.tile([128, D], f32, tag="prodK")
                    prod1 = scr.tile([128, KD], f32, tag="prod1")
                    for n in range(NT):
                        nc.vector.scalar_tensor_tensor(
                            out=prodK,
                            in0=xK_f32[:, n, :],
                            scalar=0.0,
                            in1=st["vb"][:, 0:D],
                            op0=mybir.AluOpType.bypass,
                            op1=mybir.AluOpType.mult,
                            accum_out=sA[:, n : n + 1],
                        )
                        nc.vector.scalar_tensor_tensor(
                            out=prod1,
                            in0=xk1_f32[:, n, :],
                            scalar=0.0,
                            in1=st["vb"][:, D:F],
                            op0=mybir.AluOpType.bypass,
                            op1=mybir.AluOpType.mult,
                            accum_out=sB[:, n : n + 1],
                        )
                    nc.vector.tensor_add(sB, sA, sB)
                    p_raw = small.tile([128, NT], f32, tag="p_raw")
                    nc.scalar.activation(out=p_raw, in_=sB, func=mybir.ActivationFunctionType.Exp)

                    # pp[:, n, :]: col b = p*adj*sm, col 4+b = p*adj*(1-sm), rest 0
                    pp = small.tile([128, NT, 8], mm_dt, tag="pp")
                    nc.vector.tensor_mul(pp, zf, zf)
                    nc.vector.tensor_mul(pp[:, :, b], p_raw, st["m0"][:, b, :])
                    nc.vector.tensor_mul(pp[:, :, 4 + b], p_raw, st["m1"][:, b, :])

                    for n in range(NT):
                        first = b == 0 and n == 0
                        last = b == BL - 1 and n == NT - 1
                        nc.tensor.matmul(psK, pp[:, n, :], xK[:, n, :], start=first, stop=last)
                        # k1 accumulate + softmax-denominator ones column share
                        # one PSUM group (partial-width writes accumulate fine)
                        nc.tensor.matmul(
                            ps1[:, 0:KD], pp[:, n, :], xk1[:, n, :], start=first, stop=False
                        )
                        nc.tensor.matmul(
                            ps1[:, KD : KD + 2],
                            pp[:, n, :],
                            ones2,
                            start=False,
                            stop=last,
                        )

                # ---- finishing: out = (U0@G0 + U1@G1) / P ------------------
                uall = uallp.tile([8, F + 1], f32)
                nc.vector.tensor_copy(uall[:, 0:D], psK)
                nc.vector.tensor_copy(uall[:, D : F + 1], ps1[:, 0 : KD + 1])

                uallT = uallTp.tile([128, 7, 8], f32)
                for k in range(6):
                    trp = psTr.tile([128, 8], f32)
                    nc.tensor.transpose(trp, uall[:, k * 128 : (k + 1) * 128], ident[0:8, 0:8])
                    nc.vector.tensor_copy(uallT[:, k, :], trp)
                trp = psTr.tile([128, 8], f32)
                nc.tensor.transpose(trp[0:1, :], uall[:, F : F + 1], ident[0:8, 0:8])
                nc.vector.tensor_copy(uallT[0:1, 6, :], trp[0:1, :])

                po = psOut.tile([4, D + 1], f32)  # cols 0:D main, col D = P (bank 2)
                for k in range(6):
                    nc.tensor.matmul(
                        po[:, 0:D], uallT[:, k, 0:4], st["G0"][:, k, :], start=(k == 0), stop=False
                    )
                nc.tensor.matmul(
                    po[:, 0:D], uallT[0:1, 6, 0:4], st["G0"][0:1, 6, :], start=False, stop=False
                )
                for k in range(6):
                    nc.tensor.matmul(
                        po[:, 0:D], uallT[:, k, 4:8], st["G1"][:, k, :], start=False, stop=False
                    )
                nc.tensor.matmul(
                    po[:, 0:D], uallT[0:1, 6, 4:8], st["G1"][0:1, 6, :], start=False, stop=True
                )
                nc.tensor.matmul(po[:, D : D + 1], uallT[0:1, 6, 0:4], ones11, start=True, stop=False)
                nc.tensor.matmul(po[:, D : D + 1], uallT[0:1, 6, 4:8], ones11, start=False, stop=True)

                rp = finp.tile([4, 1], f32, tag="rp")
                nc.vector.reciprocal(rp, po[:, D : D + 1])
                osb = finp.tile([4, D], f32, tag="osb")
                nc.vector.tensor_scalar_mul(out=osb, in0=po[:, 0:D], scalar1=rp)
                nc.sync.dma_start(out=br["out"][:, :], in_=osb)

    nc.compile()
    return nc


def _get_nc(stream_f32r: bool):
    key = ("nc", stream_f32r)
    if key not in _BUILD_CACHE:
        _BUILD_CACHE[key] = _build(stream_f32r)
    return _BUILD_CACHE[key]


def kernel(**inputs) -> tuple:
    global last_results
    from concourse.bass_utils import run_bass_kernel_spmd

    f32 = np.float32
    K = np.ascontiguousarray(np.asarray(inputs["K"], dtype=f32))
    front_k1 = np.ascontiguousarray(np.asarray(inputs["front_k1"], dtype=f32))
    back_K = np.ascontiguousarray(np.asarray(inputs["back_K"], dtype=f32))
    back_k2 = np.ascontiguousarray(np.asarray(inputs["back_k2"], dtype=f32))
    Wfk = np.asarray(inputs["Wfk"], dtype=f32)
    bfk = np.asarray(inputs["bfk"], dtype=f32)
    Wbk = np.asarray(inputs["Wbk"], dtype=f32)
    bbk = np.asarray(inputs["bbk"], dtype=f32)
    Wr0 = np.asarray(inputs["Wr0"], dtype=f32)
    Wr1 = np.asarray(inputs["Wr1"], dtype=f32)
    wf_den = np.asarray(inputs["wf_den"], dtype=f32)
    wb_den = np.asarray(inputs["wb_den"], dtype=f32)
    adj_f = np.ascontiguousarray(np.asarray(inputs["front_sdj_den"], dtype=np.int32))
    sm_f = np.ascontiguousarray(np.asarray(inputs["front_s_mask"], dtype=np.int32))
    adj_b = np.ascontiguousarray(np.asarray(inputs["back_sdj_den"], dtype=np.int32))
    sm_b = np.ascontiguousarray(np.asarray(inputs["back_s_mask"], dtype=np.int32))
    i = int(np.asarray(inputs["i"]))
    num_utter = int(np.asarray(inputs["num_utter"]))

    # host-folded weights
    v_f = (Wfk.astype(np.float64) @ wf_den[D:].astype(np.float64)).astype(f32)
    v_b = (Wbk.astype(np.float64) @ wb_den[D:].astype(np.float64)).astype(f32)
    A_f = np.vstack([Wfk, bfk[None, :]]).astype(np.float64)
    A_b = np.vstack([Wbk, bbk[None, :]]).astype(np.float64)
    G0_f = (A_f @ Wr0.astype(np.float64)).astype(f32)
    G1_f = (A_f @ Wr1.astype(np.float64)).astype(f32)
    G0_b = (A_b @ Wr0.astype(np.float64)).astype(f32)
    G1_b = (A_b @ Wr1.astype(np.float64)).astype(f32)

    stream_f32r = os.environ.get("KERNEL_MM_F32R", "1") == "1"
    nc = _get_nc(stream_f32r)

    in_maps = []
    for c in range(NCORES):
        s = slice(c * BL, (c + 1) * BL)
        in_maps.append(
            {
                "xK_f": K[s],
                "xk1_f": front_k1[s],
                "xK_b": back_K[s],
                "xk1_b": back_k2[s],
                "adj_f": adj_f[s],
                "sm_f": sm_f[s],
                "adj_b": adj_b[s],
                "sm_b": sm_b[s],
                "v_f": v_f,
                "v_b": v_b,
                "G0_f": G0_f,
                "G1_f": G1_f,
                "G0_b": G0_b,
                "G1_b": G1_b,
            }
        )

    trace = os.environ.get("KERNEL_TRACE", "0") == "1"
    res = run_bass_kernel_spmd(nc, in_maps, core_ids=list(range(NCORES)), trace=trace)
    last_results = res

    front = np.concatenate([r["out_f"] for r in res.results], axis=0)
    back = np.concatenate([r["out_b"] for r in res.results], axis=0)
    if i == 0:
        front = np.zeros((B, D), dtype=f32)
    if i == num_utter - 1:
        back = np.zeros((B, D), dtype=f32)
    return (front, back)



# revision 9
# speedup vs baseline: 2.2322x; 2.2322x over previous
"""Trainium2 Bass kernel for the DialogGCN GAT-style message-passing layer.

Math notes (why this is much cheaper than the reference graph):
  Kp    = concat(K, kfeat) @ Wk + bk                    (B,N,D)
  alpha = Q@wden[:D] + Kp@wden[D:] + bden               (B,N)
  w     = softmax(alpha - (1-adj)*1e30, axis=N)
  out   = sum_n w * ((Kp@Wr0)*sm + (Kp@Wr1)*(1-sm))

* softmax is invariant to per-row constants, so the Q term, bden and the
  bk@wden[D:] constant all cancel:  w = softmax_n(X_n . v) masked, where
  X = concat(K, kfeat) and v = Wk @ wden[D:]  (folded on host).
* the output is linear in the weighted sums:
    out = (sum_n w*sm*X_n | P0) @ [Wk;bk] @ Wr0 + (sum_n w*(1-sm)*X_n | P1) @ [Wk;bk] @ Wr1
  so G0 = [Wk;bk]@Wr0 and G1 = [Wk;bk]@Wr1 are folded on host (769x512 each)
  and the device only needs one streaming pass over X computing
    s_n = X_n . v ; p_n = exp(s_n)*adj_n ; U0 = sum p*sm*[X|1] ; U1 = sum p*(1-sm)*[X|1]
  followed by a tiny projection (U0@G0 + U1@G1) / P.
* masked tokens (adj=0) contribute exactly zero (the reference's -1e30 shift
  underflows exp to 0.0), so the host compacts each row's tokens down to the
  adj=1 subset, padded to C = ceil(max_count/128)*128.  The device streams
  C tokens instead of N=2048.  Streams/weights ship as fp16 (rel-err budget
  2e-2; fp16 contributes ~1e-3).

Device pipeline per (branch, batch):
  DMA x [128, NT, 768] fp16  (SP HWDGE queue)
  scores: scalar_tensor_tensor dot-products split DVE (d<A) / Pool (d>=A)
  exp on ACT; pp = p*mask columns (DVE); per-batch softmax denominators
  accumulated via DVE STT accum into App, one 1-row matmul per branch.
  PE: per n-tile 2 fp16 matmuls accumulate U in PSUM ([8,512] + [8,256]).
  Finish: U -> transpose -> (U0@G0 + U1@G1)/P -> out DMA.

Sharding: pure data parallel over batch B=32 across 8 cores (4 rows each).
"""

import os
import sys

import numpy as np

for _p in ("/opt/trn_rl_repo", "/root/.axon_site/_ro/trn_rl_repo"):
    if os.path.isdir(_p) and _p not in sys.path:
        sys.path.insert(0, _p)

B, N, D, KD = 32, 2048, 512, 256
F = D + KD  # 768
NCORES = 8
BL = B // NCORES  # 4 batch rows per core

_BUILD_CACHE = {}
last_results = None  # BassKernelResults of the most recent run (for test.py)


def _build(NT: int, ps: int, rd: int):
    """Trace the Bass program (same NEFF runs SPMD on all 8 cores).

    NT : compacted tokens per partition (C = 128*NT context length)
    ps : score multiply columns done by the Pool engine ([F-ps:F))
    rd : score reduce columns done by DVE tensor_reduce ([0:rd); ACT
         activation-accumulate covers [rd:F))

    Score engine split (STT is DVE-only and capped at 1 elem/cycle/lane, so
    the dot-product is decomposed): DVE fp16 TT-multiply runs in 2x mode,
    Pool TT-multiply takes a slice, and the per-token sums come from one
    segmented DVE tensor_reduce plus per-n-tile ACT copy-accumulates.
    """
    import concourse.bass as bass
    import concourse.tile as tile
    from concourse import bacc, mybir
    from concourse.masks import make_identity

    f32 = mybir.dt.float32
    f16 = mybir.dt.float16
    C = 128 * NT
    MS = F - ps  # DVE multiply slice [0:MS)

    nc = bacc.Bacc()

    ins = {}
    for s in ("f", "b"):
        ins[f"x_{s}"] = nc.dram_tensor(f"x_{s}", [BL, C, F], f16, kind="ExternalInput")
        # masks pre-laid-out on host to the SBUF layout (token = p*NT + n)
        ins[f"m0_{s}"] = nc.dram_tensor(f"m0_{s}", [128, BL, NT], f16, kind="ExternalInput")
        ins[f"m1_{s}"] = nc.dram_tensor(f"m1_{s}", [128, BL, NT], f16, kind="ExternalInput")
        ins[f"v_{s}"] = nc.dram_tensor(f"v_{s}", [F], f16, kind="ExternalInput")
        ins[f"G0_{s}"] = nc.dram_tensor(f"G0_{s}", [F + 1, D], f16, kind="ExternalInput")
        ins[f"G1_{s}"] = nc.dram_tensor(f"G1_{s}", [F + 1, D], f16, kind="ExternalInput")
        ins[f"out_{s}"] = nc.dram_tensor(f"out_{s}", [BL, D], f32, kind="ExternalOutput")

    with tile.TileContext(nc) as tc:
        with (
            tc.tile_pool(name="singles", bufs=1) as singles,
            tc.tile_pool(name="xp", bufs=6) as xp,
            tc.tile_pool(name="scr", bufs=3) as scr,
            tc.tile_pool(name="small", bufs=4) as small,
            tc.tile_pool(name="ppp", bufs=3) as ppp,
            tc.tile_pool(name="appp", bufs=2) as appp,
            tc.tile_pool(name="uallp", bufs=2) as uallp,
            tc.tile_pool(name="uallTp", bufs=2) as uallTp,
            tc.tile_pool(name="finp", bufs=2) as finp,
            tc.tile_pool(name="psU_K", bufs=1, space="PSUM") as psU_K,
            tc.tile_pool(name="psU_1", bufs=1, space="PSUM") as psU_1,
            tc.tile_pool(name="psPp", bufs=1, space="PSUM") as psPp,
            tc.tile_pool(name="psTr", bufs=2, space="PSUM") as psTr,
            tc.tile_pool(name="psOut", bufs=2, space="PSUM") as psOut,
        ):
            # ---- one-time setup -------------------------------------------
            identh = singles.tile([128, 128], f16)
            make_identity(nc, identh)
            ones1f = singles.tile([128, 1], f32)
            nc.gpsimd.memset(ones1f, 1.0)
            ones11h = singles.tile([1, 1], f16)
            nc.gpsimd.memset(ones11h, 1.0)

            st = {}
            for s in ("f", "b"):
                d = {}
                vb = singles.tile([128, F], f16, tag=f"vb_{s}")
                vap = ins[f"v_{s}"][:]
                nc.scalar.dma_start(
                    out=vb,
                    in_=bass.AP(tensor=vap.tensor, offset=vap.offset, ap=[[0, 128]] + vap.ap),
                )
                d["vb"] = vb
                m0s = singles.tile([128, BL, NT], f16, tag=f"m0_{s}")
                m1s = singles.tile([128, BL, NT], f16, tag=f"m1_{s}")
                nc.scalar.dma_start(out=m0s, in_=ins[f"m0_{s}"][:, :, :])
                nc.scalar.dma_start(out=m1s, in_=ins[f"m1_{s}"][:, :, :])
                d["m0"], d["m1"] = m0s, m1s
                st[s] = d

            def load_G(s, which):
                # G matrices: (128, 7, 512); chunk 6 row 0 holds row 768.
                # Issued mid-pipeline (ACT queue) so the 3.2MB of weights
                # doesn't compete with the first token streams for DMA.
                g = ins[f"G{which}_{s}"]
                gs = singles.tile([128, 7, D], f16, tag=f"G{which}_{s}")
                nc.scalar.dma_start(
                    out=gs[:, 0:6, :],
                    in_=g[0:F, :].rearrange("(k p) n -> p k n", p=128),
                )
                nc.scalar.dma_start(out=gs[0:1, 6, :], in_=g[F : F + 1, :])
                st[s][f"G{which}"] = gs

            # ---- streaming + finishing per branch -------------------------
            for s in ("f", "b"):
                d = st[s]
                psK = psU_K.tile([8, D], f32)  # rows 0-3: U0(b), rows 4-7: U1(b)
                ps1 = psU_1.tile([8, KD], f32)
                App = appp.tile([128, 8], f32)  # per-partition softmax-denominator partials

                for b in range(BL):
                    x = xp.tile([128, NT, F], f16, tag="x")
                    nc.sync.dma_start(
                        out=x, in_=ins[f"x_{s}"][b].rearrange("(p n) d -> p n d", n=NT)
                    )

                    sA = small.tile([128, NT], f32, tag="sA")
                    sP = small.tile([128, NT], f32, tag="sP")
                    prodS = scr.tile([128, NT, F], f16, tag="prodS")
                    junkS = scr.tile([128, F - rd], f16, tag="junkS")
                    for n in range(NT):
                        # elementwise x*v products; DVE runs these in 2x mode
                        nc.vector.tensor_mul(
                            prodS[:, n, 0:MS], x[:, n, 0:MS], d["vb"][:, 0:MS]
                        )
                        if ps:
                            nc.gpsimd.tensor_mul(
                                prodS[:, n, MS:F], x[:, n, MS:F], d["vb"][:, MS:F]
                            )
                        # per-token partial sums over [rd:F) on the ACT engine
                        nc.scalar.activation(
                            out=junkS,
                            in_=prodS[:, n, rd:F],
                            func=mybir.ActivationFunctionType.Copy,
                            accum_out=sP[:, n : n + 1],
                        )
                    # segmented sum over [0:rd) in one DVE instruction
                    nc.vector.tensor_reduce(
                        out=sA,
                        in_=prodS[:, :, 0:rd],
                        axis=mybir.AxisListType.X,
                        op=mybir.AluOpType.add,
                    )
                    sS = small.tile([128, NT], f32, tag="sS")
                    nc.vector.tensor_add(sS, sA, sP)
                    p_raw = small.tile([128, NT], f32, tag="p_raw")
                    nc.scalar.activation(out=p_raw, in_=sS, func=mybir.ActivationFunctionType.Exp)

                    # pp[:, n, :]: col b = p*m0, col 4+b = p*m1, rest 0
                    pp = ppp.tile([128, NT, 8], f16, tag="pp")
                    nc.vector.memset(pp, 0.0)
                    nc.vector.tensor_mul(pp[:, :, b], p_raw, d["m0"][:, b, :])
                    nc.vector.tensor_mul(pp[:, :, 4 + b], p_raw, d["m1"][:, b, :])
                    # per-partition denominator partials for this batch
                    junk = scr.tile([128, NT], f16, tag="junk")
                    nc.vector.scalar_tensor_tensor(
                        out=junk,
                        in0=p_raw,
                        scalar=0.0,
                        in1=d["m0"][:, b, :],
                        op0=mybir.AluOpType.bypass,
                        op1=mybir.AluOpType.mult,
                        accum_out=App[:, b : b + 1],
                    )
                    junk2 = scr.tile([128, NT], f16, tag="junk2")
                    nc.vector.scalar_tensor_tensor(
                        out=junk2,
                        in0=p_raw,
                        scalar=0.0,
                        in1=d["m1"][:, b, :],
                        op0=mybir.AluOpType.bypass,
                        op1=mybir.AluOpType.mult,
                        accum_out=App[:, 4 + b : 4 + b + 1],
                    )

                    for n in range(NT):
                        first = b == 0 and n == 0
                        last = b == BL - 1 and n == NT - 1
                        nc.tensor.matmul(psK, pp[:, n, :], x[:, n, 0:D], start=first, stop=last)
                        nc.tensor.matmul(ps1, pp[:, n, :], x[:, n, D:F], start=first, stop=last)

                    # stagger the weight loads into the pipeline
                    if s == "f" and b == 1:
                        load_G("f", 0)
                        load_G("f", 1)
                    if s == "f" and b == 3:
                        load_G("b", 0)
                        load_G("b", 1)

                # ---- finishing: out = (U0@G0 + U1@G1) / P ------------------
                psP8 = psPp.tile([8, 1], f32, tag="psP")
                nc.tensor.matmul(psP8, App, ones1f, start=True, stop=True)

                uall = uallp.tile([8, F + 1], f16)
                nc.vector.tensor_copy(uall[:, 0:D], psK)
                nc.vector.tensor_copy(uall[:, D:F], ps1)
                nc.vector.tensor_copy(uall[:, F : F + 1], psP8)

                uallT = uallTp.tile([128, 7, 8], f16)
                for k in range(6):
                    trp = psTr.tile([128, 8], f16)
                    nc.tensor.transpose(trp, uall[:, k * 128 : (k + 1) * 128], identh[0:8, 0:8])
                    nc.vector.tensor_copy(uallT[:, k, :], trp)
                trp = psTr.tile([128, 8], f16)
                nc.tensor.transpose(trp[0:1, :], uall[:, F : F + 1], identh[0:8, 0:8])
                nc.vector.tensor_copy(uallT[0:1, 6, :], trp[0:1, :])

                po = psOut.tile([4, D], f32)
                for k in range(6):
                    nc.tensor.matmul(
                        po, uallT[:, k, 0:4], d["G0"][:, k, :], start=(k == 0), stop=False
                    )
                nc.tensor.matmul(po, uallT[0:1, 6, 0:4], d["G0"][0:1, 6, :], start=False, stop=False)
                for k in range(6):
                    nc.tensor.matmul(po, uallT[:, k, 4:8], d["G1"][:, k, :], start=False, stop=False)
                nc.tensor.matmul(po, uallT[0:1, 6, 4:8], d["G1"][0:1, 6, :], start=False, stop=True)

                psP4 = psPp.tile([4, 1], f32, tag="psP")
                nc.tensor.matmul(psP4, uallT[0:1, 6, 0:4], ones11h, start=True, stop=False)
                nc.tensor.matmul(psP4, uallT[0:1, 6, 4:8], ones11h, start=False, stop=True)

                rp = finp.tile([4, 1], f32, tag="rp")
                nc.vector.reciprocal(rp, psP4)
                osb = finp.tile([4, D], f32, tag="osb")
                nc.vector.tensor_scalar_mul(out=osb, in0=po, scalar1=rp)
                d["osb"] = osb

            # outputs issued at program end on the SP queue: by then all
            # stream dma_starts are already enqueued, so the dep waits here
            # cannot stall anything
            for s in ("f", "b"):
                nc.sync.dma_start(out=ins[f"out_{s}"][:, :], in_=st[s]["osb"])

    nc.compile()
    return nc


def _get_nc(NT: int, ps: int, rd: int):
    key = (NT, ps, rd)
    if key not in _BUILD_CACHE:
        _BUILD_CACHE[key] = _build(NT, ps, rd)
    return _BUILD_CACHE[key]


def _compact(Kv, k1v, adj, sm, C):
    """Per batch row: gather adj=1 tokens of concat(K, k1); build m0/m1."""
    f16 = np.float16
    nb = Kv.shape[0]
    xc = np.zeros((nb, C, F), dtype=f16)
    m0 = np.zeros((nb, C), dtype=f16)
    m1 = np.zeros((nb, C), dtype=f16)
    for g in range(nb):
        idx = np.flatnonzero(adj[g])
        k = len(idx)
        xc[g, :k, 0:D] = Kv[g, idx]
        xc[g, :k, D:F] = k1v[g, idx]
        smg = sm[g, idx].astype(f16)
        m0[g, :k] = smg
        m1[g, :k] = 1.0 - smg
    return xc, m0, m1


def kernel(**inputs) -> tuple:
    global last_results
    from concourse.bass_utils import run_bass_kernel_spmd

    f32 = np.float32
    f16 = np.float16
    K = np.asarray(inputs["K"], dtype=f32)
    front_k1 = np.asarray(inputs["front_k1"], dtype=f32)
    back_K = np.asarray(inputs["back_K"], dtype=f32)
    back_k2 = np.asarray(inputs["back_k2"], dtype=f32)
    Wfk = np.asarray(inputs["Wfk"], dtype=f32)
    bfk = np.asarray(inputs["bfk"], dtype=f32)
    Wbk = np.asarray(inputs["Wbk"], dtype=f32)
    bbk = np.asarray(inputs["bbk"], dtype=f32)
    Wr0 = np.asarray(inputs["Wr0"], dtype=f32)
    Wr1 = np.asarray(inputs["Wr1"], dtype=f32)
    wf_den = np.asarray(inputs["wf_den"], dtype=f32)
    wb_den = np.asarray(inputs["wb_den"], dtype=f32)
    adj_f = np.asarray(inputs["front_sdj_den"], dtype=np.int32)
    sm_f = np.asarray(inputs["front_s_mask"], dtype=np.int32)
    adj_b = np.asarray(inputs["back_sdj_den"], dtype=np.int32)
    sm_b = np.asarray(inputs["back_s_mask"], dtype=np.int32)
    i = int(np.asarray(inputs["i"]))
    num_utter = int(np.asarray(inputs["num_utter"]))

    # host-folded weights
    v_f = (Wfk.astype(np.float64) @ wf_den[D:].astype(np.float64)).astype(f16)
    v_b = (Wbk.astype(np.float64) @ wb_den[D:].astype(np.float64)).astype(f16)
    A_f = np.vstack([Wfk, bfk[None, :]]).astype(np.float64)
    A_b = np.vstack([Wbk, bbk[None, :]]).astype(np.float64)
    G0_f = (A_f @ Wr0.astype(np.float64)).astype(f16)
    G1_f = (A_f @ Wr1.astype(np.float64)).astype(f16)
    G0_b = (A_b @ Wr0.astype(np.float64)).astype(f16)
    G1_b = (A_b @ Wr1.astype(np.float64)).astype(f16)

    # context length after compaction (adj=0 tokens contribute exactly 0)
    maxcnt = max(int(adj_f.sum(axis=1).max()), int(adj_b.sum(axis=1).max()), 1)
    C = min(N, ((maxcnt + 127) // 128) * 128)
    NT = C // 128

    xc_f, m0_f, m1_f = _compact(K, front_k1, adj_f, sm_f, C)
    xc_b, m0_b, m1_b = _compact(back_K, back_k2, adj_b, sm_b, C)

    ps = int(os.environ.get("KERNEL_PS", "192"))
    rd = int(os.environ.get("KERNEL_RD", "224"))
    nc = _get_nc(NT, ps, rd)

    def mask_layout(m):
        # [BL, C] -> [128, BL, NT] matching token = p*NT + n
        return np.ascontiguousarray(m.reshape(BL, 128, NT).transpose(1, 0, 2))

    in_maps = []
    for c in range(NCORES):
        sl = slice(c * BL, (c + 1) * BL)
        in_maps.append(
            {
                "x_f": np.ascontiguousarray(xc_f[sl]),
                "x_b": np.ascontiguousarray(xc_b[sl]),
                "m0_f": mask_layout(m0_f[sl]),
                "m1_f": mask_layout(m1_f[sl]),
                "m0_b": mask_layout(m0_b[sl]),
                "m1_b": mask_layout(m1_b[sl]),
                "v_f": v_f,
                "v_b": v_b,
                "G0_f": G0_f,
                "G1_f": G1_f,
                "G0_b": G0_b,
                "G1_b": G1_b,
            }
        )

    trace = os.environ.get("KERNEL_TRACE", "0") == "1"
    res = run_bass_kernel_spmd(nc, in_maps, core_ids=list(range(NCORES)), trace=trace)
    last_results = res

    front = np.concatenate([r["out_f"] for r in res.results], axis=0)
    back = np.concatenate([r["out_b"] for r in res.results], axis=0)
    if i == 0:
        front = np.zeros((B, D), dtype=f32)
    if i == num_utter - 1:
        back = np.zeros((B, D), dtype=f32)
    return (front, back)


# revision 16
# speedup vs baseline: 2.3712x; 1.0623x over previous
"""Trainium2 Bass kernel for the DialogGCN GAT-style message-passing layer.

Math notes (why this is much cheaper than the reference graph):
  Kp    = concat(K, kfeat) @ Wk + bk                    (B,N,D)
  alpha = Q@wden[:D] + Kp@wden[D:] + bden               (B,N)
  w     = softmax(alpha - (1-adj)*1e30, axis=N)
  out   = sum_n w * ((Kp@Wr0)*sm + (Kp@Wr1)*(1-sm))

* softmax is invariant to per-row constants, so the Q term, bden and the
  bk@wden[D:] constant all cancel:  w = softmax_n(X_n . v) masked, where
  X = concat(K, kfeat) and v = Wk @ wden[D:]  (folded on host).
* the output is linear in the weighted sums:
    out = (sum_n w*sm*X_n | P0) @ [Wk;bk] @ Wr0 + (sum_n w*(1-sm)*X_n | P1) @ [Wk;bk] @ Wr1
  so G0 = [Wk;bk]@Wr0 and G1 = [Wk;bk]@Wr1 are folded on host (769x512 each)
  and the device only needs one streaming pass over X computing
    s_n = X_n . v ; p_n = exp(s_n)*adj_n ; U0 = sum p*sm*[X|1] ; U1 = sum p*(1-sm)*[X|1]
  followed by a tiny projection (U0@G0 + U1@G1) / P.
* masked tokens (adj=0) contribute exactly zero (the reference's -1e30 shift
  underflows exp to 0.0), so the host compacts each row's tokens down to the
  adj=1 subset, padded to C = ceil(max_count/128)*128.  The device streams
  C tokens instead of N=2048.  Streams/weights ship as fp16 (rel-err budget
  2e-2; fp16 contributes ~1e-3).

Device pipeline per (branch, batch):
  DMA x [128, NT, 768] fp16  (SP HWDGE queue)
  scores: scalar_tensor_tensor dot-products split DVE (d<A) / Pool (d>=A)
  exp on ACT; pp = p*mask columns (DVE); per-batch softmax denominators
  accumulated via DVE STT accum into App, one 1-row matmul per branch.
  PE: per n-tile 2 fp16 matmuls accumulate U in PSUM ([8,512] + [8,256]).
  Finish: U -> transpose -> (U0@G0 + U1@G1)/P -> out DMA.

Sharding: pure data parallel over batch B=32 across 8 cores (4 rows each).
"""

import os
import sys

import numpy as np

for _p in ("/opt/trn_rl_repo", "/root/.axon_site/_ro/trn_rl_repo"):
    if os.path.isdir(_p) and _p not in sys.path:
        sys.path.insert(0, _p)

B, N, D, KD = 32, 2048, 512, 256
F = D + KD  # 768
NCORES = 8
BL = B // NCORES  # 4 batch rows per core

_BUILD_CACHE = {}
last_results = None  # BassKernelResults of the most recent run (for test.py)


def _build(NT: int, ps: int, rn: int):
    """Trace the Bass program (same NEFF runs SPMD on all 8 cores).

    NT : compacted tokens per partition (C = 128*NT context length)
    ps : score multiply columns done by the Pool engine ([F-ps:F))
    rn : score n-tiles whose row-sum is done by DVE tensor_reduce ([0:rn));
         the ACT engine covers n in [rn:NT) with full-row copy-accumulates

    Score engine split (measured DVE fixed cost is ~260ns/instruction and
    ACT's accumulate costs ~525ns fixed, so ops are fused where possible):
    one DVE fp16 TT-multiply in 2x mode + one Pool TT-multiply produce the
    x*v products for all n; per-token sums come from one segmented DVE
    tensor_reduce (n < rn) plus per-n ACT copy-accumulates (n >= rn).
    """
    import concourse.bass as bass
    import concourse.tile as tile
    from concourse import bacc, mybir
    from concourse.masks import make_identity

    f32 = mybir.dt.float32
    f16 = mybir.dt.float16
    C = 128 * NT
    MS = F - ps  # DVE multiply slice [0:MS)

    nc = bacc.Bacc()

    ins = {}
    for s in ("f", "b"):
        ins[f"x_{s}"] = nc.dram_tensor(f"x_{s}", [BL, C, F], f16, kind="ExternalInput")
        # masks pre-laid-out on host to the SBUF layout (token = p*NT + n)
        ins[f"m0_{s}"] = nc.dram_tensor(f"m0_{s}", [128, BL, NT], f16, kind="ExternalInput")
        ins[f"m1_{s}"] = nc.dram_tensor(f"m1_{s}", [128, BL, NT], f16, kind="ExternalInput")
        ins[f"v_{s}"] = nc.dram_tensor(f"v_{s}", [F], f16, kind="ExternalInput")
        ins[f"G0_{s}"] = nc.dram_tensor(f"G0_{s}", [F + 1, D], f16, kind="ExternalInput")
        ins[f"G1_{s}"] = nc.dram_tensor(f"G1_{s}", [F + 1, D], f16, kind="ExternalInput")
        ins[f"out_{s}"] = nc.dram_tensor(f"out_{s}", [BL, D], f32, kind="ExternalOutput")

    with tile.TileContext(nc) as tc:
        with (
            tc.tile_pool(name="singles", bufs=1) as singles,
            tc.tile_pool(name="xp", bufs=6) as xp,
            tc.tile_pool(name="scr", bufs=3) as scr,
            tc.tile_pool(name="small", bufs=4) as small,
            tc.tile_pool(name="ppp", bufs=3) as ppp,
            tc.tile_pool(name="appp", bufs=2) as appp,
            tc.tile_pool(name="uallp", bufs=2) as uallp,
            tc.tile_pool(name="uallTp", bufs=2) as uallTp,
            tc.tile_pool(name="finp", bufs=2) as finp,
            tc.tile_pool(name="psU_K", bufs=1, space="PSUM") as psU_K,
            tc.tile_pool(name="psU_1", bufs=1, space="PSUM") as psU_1,
            tc.tile_pool(name="psPp", bufs=1, space="PSUM") as psPp,
            tc.tile_pool(name="psTr", bufs=2, space="PSUM") as psTr,
            tc.tile_pool(name="psOut", bufs=2, space="PSUM") as psOut,
        ):
            # ---- one-time setup -------------------------------------------
            identh = singles.tile([128, 128], f16)
            make_identity(nc, identh)
            ones1f = singles.tile([128, 1], f32)
            nc.gpsimd.memset(ones1f, 1.0)
            ones11h = singles.tile([1, 1], f16)
            nc.gpsimd.memset(ones11h, 1.0)

            st = {}
            for s in ("f", "b"):
                d = {}
                vb = singles.tile([128, F], f16, tag=f"vb_{s}")
                vap = ins[f"v_{s}"][:]
                nc.scalar.dma_start(
                    out=vb,
                    in_=bass.AP(tensor=vap.tensor, offset=vap.offset, ap=[[0, 128]] + vap.ap),
                )
                d["vb"] = vb
                m0s = singles.tile([128, BL, NT], f16, tag=f"m0_{s}")
                m1s = singles.tile([128, BL, NT], f16, tag=f"m1_{s}")
                nc.scalar.dma_start(out=m0s, in_=ins[f"m0_{s}"][:, :, :])
                nc.scalar.dma_start(out=m1s, in_=ins[f"m1_{s}"][:, :, :])
                d["m0"], d["m1"] = m0s, m1s
                st[s] = d

            def load_G(s, which):
                # G matrices: (128, 7, 512); chunk 6 row 0 holds row 768.
                # Issued mid-pipeline (ACT queue) so the 3.2MB of weights
                # doesn't compete with the first token streams for DMA.
                g = ins[f"G{which}_{s}"]
                gs = singles.tile([128, 7, D], f16, tag=f"G{which}_{s}")
                nc.scalar.dma_start(
                    out=gs[:, 0:6, :],
                    in_=g[0:F, :].rearrange("(k p) n -> p k n", p=128),
                )
                nc.scalar.dma_start(out=gs[0:1, 6, :], in_=g[F : F + 1, :])
                st[s][f"G{which}"] = gs

            def bcast_n(ap2d, lo, hi, cnt):
                # [128, K] slice -> [128, cnt, K] with a stride-0 middle dim
                sl = ap2d[:, lo:hi]
                return bass.AP(
                    tensor=sl.tensor, offset=sl.offset, ap=[sl.ap[0], [0, cnt], sl.ap[1]]
                )

            # ---- streaming + finishing per branch -------------------------
            for si, s in enumerate(("f", "b")):
                d = st[s]
                psK = psU_K.tile([8, D], f32)  # rows 0-3: U0(b), rows 4-7: U1(b)
                ps1 = psU_1.tile([8, KD], f32)
                App = appp.tile([128, 8], f32)  # per-partition softmax-denominator partials

                for b in range(BL):
                    first_bb = si == 0 and b == 0
                    last_bb = si == 1 and b == BL - 1
                    # chunk the fill/drain iterations so downstream engines
                    # start earlier (first) / drain sooner (last)
                    n_chunks = (
                        [(0, 3), (3, 6), (6, NT)] if (first_bb or last_bb) else [(0, NT)]
                    )

                    x = xp.tile([128, NT, F], f16, tag="x")
                    xsrc = ins[f"x_{s}"][b].rearrange("(p n) d -> p n d", n=NT)
                    for lo, hi in n_chunks:
                        nc.sync.dma_start(out=x[:, lo:hi, :], in_=xsrc[:, lo:hi, :])

                    prodS = scr.tile([128, NT, F], f16, tag="prodS")
                    junkS = scr.tile([128, F], f16, tag="junkS")
                    sS = small.tile([128, NT], f32, tag="sS")
                    for lo, hi in n_chunks:
                        # elementwise x*v products; DVE runs fp16 in 2x mode
                        nc.vector.tensor_mul(
                            prodS[:, lo:hi, 0:MS],
                            x[:, lo:hi, 0:MS],
                            bcast_n(d["vb"], 0, MS, hi - lo),
                        )
                        if ps:
                            nc.gpsimd.tensor_mul(
                                prodS[:, lo:hi, MS:F],
                                x[:, lo:hi, MS:F],
                                bcast_n(d["vb"], MS, F, hi - lo),
                            )
                    # per-token sums: DVE handles n<rn in one segmented reduce,
                    # ACT accumulates full rows for n>=rn
                    nc.vector.tensor_reduce(
                        out=sS[:, 0:rn],
                        in_=prodS[:, 0:rn, :],
                        axis=mybir.AxisListType.X,
                        op=mybir.AluOpType.add,
                    )
                    for n in range(rn, NT):
                        nc.scalar.activation(
                            out=junkS,
                            in_=prodS[:, n, :],
                            func=mybir.ActivationFunctionType.Copy,
                            accum_out=sS[:, n : n + 1],
                        )
                    p_raw = small.tile([128, NT], f32, tag="p_raw")
                    nc.scalar.activation(out=p_raw, in_=sS, func=mybir.ActivationFunctionType.Exp)

                    # pp[:, n, :]: col b = p*m0, col 4+b = p*m1, rest 0
                    pp = ppp.tile([128, NT, 8], f16, tag="pp")
                    nc.gpsimd.memset(pp, 0.0)
                    nc.vector.tensor_mul(pp[:, :, b], p_raw, d["m0"][:, b, :])
                    nc.gpsimd.tensor_mul(pp[:, :, 4 + b], p_raw, d["m1"][:, b, :])
                    # per-partition denominator partials for this batch
                    junk = scr.tile([128, NT], f16, tag="junk")
                    nc.vector.scalar_tensor_tensor(
                        out=junk,
                        in0=p_raw,
                        scalar=0.0,
                        in1=d["m0"][:, b, :],
                        op0=mybir.AluOpType.bypass,
                        op1=mybir.AluOpType.mult,
                        accum_out=App[:, b : b + 1],
                    )
                    junk2 = scr.tile([128, NT], f16, tag="junk2")
                    nc.vector.scalar_tensor_tensor(
                        out=junk2,
                        in0=p_raw,
                        scalar=0.0,
                        in1=d["m1"][:, b, :],
                        op0=mybir.AluOpType.bypass,
                        op1=mybir.AluOpType.mult,
                        accum_out=App[:, 4 + b : 4 + b + 1],
                    )

                    for n in range(NT):
                        first = b == 0 and n == 0
                        last = b == BL - 1 and n == NT - 1
                        nc.tensor.matmul(psK, pp[:, n, :], x[:, n, 0:D], start=first, stop=last)
                        nc.tensor.matmul(ps1, pp[:, n, :], x[:, n, D:F], start=first, stop=last)

                    # stagger the weight loads into the pipeline
                    if s == "f" and b == 1:
                        load_G("f", 0)
                        load_G("f", 1)
                    if s == "f" and b == 3:
                        load_G("b", 0)
                        load_G("b", 1)

                # ---- finishing: out = (U0@G0 + U1@G1) / P ------------------
                psP8 = psPp.tile([8, 1], f32, tag="psP")
                nc.tensor.matmul(psP8, App, ones1f, start=True, stop=True)

                uall = uallp.tile([8, F + 1], f16)
                nc.vector.tensor_copy(uall[:, 0:D], psK)
                nc.vector.tensor_copy(uall[:, D:F], ps1)
                nc.vector.tensor_copy(uall[:, F : F + 1], psP8)

                uallT = uallTp.tile([128, 7, 8], f16)
                for k in range(6):
                    trp = psTr.tile([128, 8], f16)
                    nc.tensor.transpose(trp, uall[:, k * 128 : (k + 1) * 128], identh[0:8, 0:8])
                    nc.vector.tensor_copy(uallT[:, k, :], trp)
                trp = psTr.tile([128, 8], f16)
                nc.tensor.transpose(trp[0:1, :], uall[:, F : F + 1], identh[0:8, 0:8])
                nc.vector.tensor_copy(uallT[0:1, 6, :], trp[0:1, :])

                po = psOut.tile([4, D], f32)
                for k in range(6):
                    nc.tensor.matmul(
                        po, uallT[:, k, 0:4], d["G0"][:, k, :], start=(k == 0), stop=False
                    )
                nc.tensor.matmul(po, uallT[0:1, 6, 0:4], d["G0"][0:1, 6, :], start=False, stop=False)
                for k in range(6):
                    nc.tensor.matmul(po, uallT[:, k, 4:8], d["G1"][:, k, :], start=False, stop=False)
                nc.tensor.matmul(po, uallT[0:1, 6, 4:8], d["G1"][0:1, 6, :], start=False, stop=True)

                psP4 = psPp.tile([4, 1], f32, tag="psP")
                nc.tensor.matmul(psP4, uallT[0:1, 6, 0:4], ones11h, start=True, stop=False)
                nc.tensor.matmul(psP4, uallT[0:1, 6, 4:8], ones11h, start=False, stop=True)

                rp = finp.tile([4, 1], f32, tag="rp")
                nc.vector.reciprocal(rp, psP4)
                osb = finp.tile([4, D], f32, tag="osb")
                nc.vector.tensor_scalar_mul(out=osb, in0=po, scalar1=rp)
                d["osb"] = osb

            # outputs issued at program end on the SP queue: by then all
            # stream dma_starts are already enqueued, so the dep waits here
            # cannot stall anything
            for s in ("f", "b"):
                nc.sync.dma_start(out=ins[f"out_{s}"][:, :], in_=st[s]["osb"])

    nc.compile()
    return nc


def _get_nc(NT: int, ps: int, rn: int):
    key = (NT, ps, rn)
    if key not in _BUILD_CACHE:
        _BUILD_CACHE[key] = _build(NT, ps, rn)
    return _BUILD_CACHE[key]


def _compact(Kv, k1v, adj, sm, C):
    """Per batch row: gather adj=1 tokens of concat(K, k1); build m0/m1."""
    f16 = np.float16
    nb = Kv.shape[0]
    xc = np.zeros((nb, C, F), dtype=f16)
    m0 = np.zeros((nb, C), dtype=f16)
    m1 = np.zeros((nb, C), dtype=f16)
    for g in range(nb):
        idx = np.flatnonzero(adj[g])
        k = len(idx)
        xc[g, :k, 0:D] = Kv[g, idx]
        xc[g, :k, D:F] = k1v[g, idx]
        smg = sm[g, idx].astype(f16)
        m0[g, :k] = smg
        m1[g, :k] = 1.0 - smg
    return xc, m0, m1


def kernel(**inputs) -> tuple:
    global last_results
    from concourse.bass_utils import run_bass_kernel_spmd

    f32 = np.float32
    f16 = np.float16
    K = np.asarray(inputs["K"], dtype=f32)
    front_k1 = np.asarray(inputs["front_k1"], dtype=f32)
    back_K = np.asarray(inputs["back_K"], dtype=f32)
    back_k2 = np.asarray(inputs["back_k2"], dtype=f32)
    Wfk = np.asarray(inputs["Wfk"], dtype=f32)
    bfk = np.asarray(inputs["bfk"], dtype=f32)
    Wbk = np.asarray(inputs["Wbk"], dtype=f32)
    bbk = np.asarray(inputs["bbk"], dtype=f32)
    Wr0 = np.asarray(inputs["Wr0"], dtype=f32)
    Wr1 = np.asarray(inputs["Wr1"], dtype=f32)
    wf_den = np.asarray(inputs["wf_den"], dtype=f32)
    wb_den = np.asarray(inputs["wb_den"], dtype=f32)
    adj_f = np.asarray(inputs["front_sdj_den"], dtype=np.int32)
    sm_f = np.asarray(inputs["front_s_mask"], dtype=np.int32)
    adj_b = np.asarray(inputs["back_sdj_den"], dtype=np.int32)
    sm_b = np.asarray(inputs["back_s_mask"], dtype=np.int32)
    i = int(np.asarray(inputs["i"]))
    num_utter = int(np.asarray(inputs["num_utter"]))

    # host-folded weights
    v_f = (Wfk.astype(np.float64) @ wf_den[D:].astype(np.float64)).astype(f16)
    v_b = (Wbk.astype(np.float64) @ wb_den[D:].astype(np.float64)).astype(f16)
    A_f = np.vstack([Wfk, bfk[None, :]]).astype(np.float64)
    A_b = np.vstack([Wbk, bbk[None, :]]).astype(np.float64)
    G0_f = (A_f @ Wr0.astype(np.float64)).astype(f16)
    G1_f = (A_f @ Wr1.astype(np.float64)).astype(f16)
    G0_b = (A_b @ Wr0.astype(np.float64)).astype(f16)
    G1_b = (A_b @ Wr1.astype(np.float64)).astype(f16)

    # context length after compaction (adj=0 tokens contribute exactly 0)
    maxcnt = max(int(adj_f.sum(axis=1).max()), int(adj_b.sum(axis=1).max()), 1)
    C = min(N, ((maxcnt + 127) // 128) * 128)
    NT = C // 128

    xc_f, m0_f, m1_f = _compact(K, front_k1, adj_f, sm_f, C)
    xc_b, m0_b, m1_b = _compact(back_K, back_k2, adj_b, sm_b, C)

    ps = int(os.environ.get("KERNEL_PS", "192"))
    rn = int(os.environ.get("KERNEL_RN", "3"))
    nc = _get_nc(NT, ps, rn)

    def mask_layout(m):
        # [BL, C] -> [128, BL, NT] matching token = p*NT + n
        return np.ascontiguousarray(m.reshape(BL, 128, NT).transpose(1, 0, 2))

    in_maps = []
    for c in range(NCORES):
        sl = slice(c * BL, (c + 1) * BL)
        in_maps.append(
            {
                "x_f": np.ascontiguousarray(xc_f[sl]),
                "x_b": np.ascontiguousarray(xc_b[sl]),
                "m0_f": mask_layout(m0_f[sl]),
                "m1_f": mask_layout(m1_f[sl]),
                "m0_b": mask_layout(m0_b[sl]),
                "m1_b": mask_layout(m1_b[sl]),
                "v_f": v_f,
                "v_b": v_b,
                "G0_f": G0_f,
                "G1_f": G1_f,
                "G0_b": G0_b,
                "G1_b": G1_b,
            }
        )

    trace = os.environ.get("KERNEL_TRACE", "0") == "1"
    res = run_bass_kernel_spmd(nc, in_maps, core_ids=list(range(NCORES)), trace=trace)
    last_results = res

    front = np.concatenate([r["out_f"] for r in res.results], axis=0)
    back = np.concatenate([r["out_b"] for r in res.results], axis=0)
    if i == 0:
        front = np.zeros((B, D), dtype=f32)
    if i == num_utter - 1:
        back = np.zeros((B, D), dtype=f32)
    return (front, back)


# revision 19
# speedup vs baseline: 2.6984x; 1.1380x over previous
"""Trainium2 Bass kernel for the DialogGCN GAT-style message-passing layer.

Math notes (why this is much cheaper than the reference graph):
  Kp    = concat(K, kfeat) @ Wk + bk                    (B,N,D)
  alpha = Q@wden[:D] + Kp@wden[D:] + bden               (B,N)
  w     = softmax(alpha - (1-adj)*1e30, axis=N)
  out   = sum_n w * ((Kp@Wr0)*sm + (Kp@Wr1)*(1-sm))

* softmax is invariant to per-row constants, so the Q term, bden and the
  bk@wden[D:] constant all cancel:  w = softmax_n(X_n . v) masked, where
  X = concat(K, kfeat) and v = Wk @ wden[D:]  (folded on host).
* the output is linear in the weighted sums:
    out = (sum w*sm*[X|1]) @ [Wk;bk] @ Wr0 + (sum w*(1-sm)*[X|1]) @ [Wk;bk] @ Wr1
  so G0 = [Wk;bk]@Wr0 and G1 = [Wk;bk]@Wr1 are folded on host (769x512 each)
  and the device only needs one streaming pass over X computing
    s_n = X_n . v ; p_n = exp(s_n)*adj_n ; U0/U1 = sum pp_n*[X|1]
  followed by a tiny projection (U0@G0 + U1@G1) / P.
* masked tokens (adj=0) contribute exactly zero (the reference's -1e30 shift
  underflows exp to 0.0), so the host packs each (core, branch)'s valid
  tokens across all 4 batch rows into one token list padded to NTB*128
  (~4224 vs 4*2048).  Batch membership and the speaker mask are carried by
  a host-built 8-column mask m8 (col b = sm, col 4+b = 1-sm for batch-b
  tokens).  A host-appended ones column (x[:,768]=1) makes the U matmul
  also produce the softmax denominators P0/P1 for free.
* streams/weights ship as fp16 (rel-err budget 2e-2; fp16 adds ~1e-3).

Device pipeline per token group (~8 tiles of 128 tokens):
  DMA x [128, g, 770] fp16  (SP HWDGE queue)
  scores: one DVE fp16 2x multiply + one Pool multiply -> prod = x*v;
  row sums: one segmented DVE tensor_reduce (n<rn) + per-n ACT
  copy-accumulates (n>=rn); ACT exp; pp = p * m8 (one broadcast multiply).
  PE: per tile 2 fp16 matmuls accumulate U in PSUM ([8,512] + [8,258]).
  Finish per branch: U -> transpose -> (U0@G0 + U1@G1)/P -> out DMA.

Sharding: pure data parallel over batch B=32 across 8 cores (4 rows each).
"""

import os
import sys

import numpy as np

for _p in ("/opt/trn_rl_repo", "/root/.axon_site/_ro/trn_rl_repo"):
    if os.path.isdir(_p) and _p not in sys.path:
        sys.path.insert(0, _p)

B, N, D, KD = 32, 2048, 512, 256
F = D + KD  # 768
FP = F + 2  # 770: ones column (768) + pad (769)
NCORES = 8
BL = B // NCORES  # 4 batch rows per core

_BUILD_CACHE = {}
last_results = None  # BassKernelResults of the most recent run (for test.py)


def _groups(NTB: int, small_first: bool):
    """Split NTB tiles into ~4 groups; a small edge group shortens the
    pipeline fill (front branch) / drain (back branch)."""
    if NTB <= 6:
        return [(0, NTB)]
    edge = min(6, NTB - 1)
    rest = NTB - edge
    ng = max(1, round(rest / 9))
    bounds = [round(rest * i / ng) for i in range(ng + 1)]
    if small_first:
        return [(0, edge)] + [(edge + a, edge + b) for a, b in zip(bounds, bounds[1:])]
    return [(a, b) for a, b in zip(bounds, bounds[1:])] + [(rest, NTB)]


def _build(NTB: int, ps: int, rn: int):
    """Trace the Bass program (same NEFF runs SPMD on all 8 cores).

    NTB : packed token tiles per branch (context = 128*NTB tokens)
    ps  : score multiply columns done by the Pool engine ([F-ps:F))
    rn  : per-group score tiles row-summed by DVE tensor_reduce; the ACT
          engine covers the rest with full-row copy-accumulates
    """
    import concourse.bass as bass
    import concourse.tile as tile
    from concourse import bacc, mybir
    from concourse.masks import make_identity

    f32 = mybir.dt.float32
    f16 = mybir.dt.float16
    CB = 128 * NTB
    MS = F - ps  # DVE multiply slice [0:MS)

    nc = bacc.Bacc()

    ins = {}
    for s in ("f", "b"):
        ins[f"x_{s}"] = nc.dram_tensor(f"x_{s}", [CB, FP], f16, kind="ExternalInput")
        ins[f"m8_{s}"] = nc.dram_tensor(f"m8_{s}", [128, NTB, 8], f16, kind="ExternalInput")
        ins[f"v_{s}"] = nc.dram_tensor(f"v_{s}", [F], f16, kind="ExternalInput")
        ins[f"G0_{s}"] = nc.dram_tensor(f"G0_{s}", [F + 1, D], f16, kind="ExternalInput")
        ins[f"G1_{s}"] = nc.dram_tensor(f"G1_{s}", [F + 1, D], f16, kind="ExternalInput")
        ins[f"out_{s}"] = nc.dram_tensor(f"out_{s}", [BL, D], f32, kind="ExternalOutput")

    with tile.TileContext(nc) as tc:
        with (
            tc.tile_pool(name="singles", bufs=1) as singles,
            tc.tile_pool(name="xp", bufs=6) as xp,
            tc.tile_pool(name="scr", bufs=3) as scr,
            tc.tile_pool(name="small", bufs=4) as small,
            tc.tile_pool(name="ppp", bufs=3) as ppp,
            tc.tile_pool(name="uallp", bufs=2) as uallp,
            tc.tile_pool(name="uallTp", bufs=2) as uallTp,
            tc.tile_pool(name="finp", bufs=2) as finp,
            tc.tile_pool(name="psU_K", bufs=1, space="PSUM") as psU_K,
            tc.tile_pool(name="psU_1", bufs=1, space="PSUM") as psU_1,
            tc.tile_pool(name="psPp", bufs=1, space="PSUM") as psPp,
            tc.tile_pool(name="psTr", bufs=2, space="PSUM") as psTr,
            tc.tile_pool(name="psOut", bufs=2, space="PSUM") as psOut,
        ):
            # ---- one-time setup -------------------------------------------
            identh = singles.tile([128, 128], f16)
            make_identity(nc, identh)
            ones11h = singles.tile([1, 1], f16)
            nc.gpsimd.memset(ones11h, 1.0)

            st = {}
            for s in ("f", "b"):
                d = {}
                vb = singles.tile([128, F], f16, tag=f"vb_{s}")
                vap = ins[f"v_{s}"][:]
                nc.scalar.dma_start(
                    out=vb,
                    in_=bass.AP(tensor=vap.tensor, offset=vap.offset, ap=[[0, 128]] + vap.ap),
                )
                d["vb"] = vb
                m8s = singles.tile([128, NTB, 8], f16, tag=f"m8_{s}")
                nc.scalar.dma_start(out=m8s, in_=ins[f"m8_{s}"][:, :, :])
                d["m8"] = m8s
                st[s] = d

            def load_G(s, which):
                # G matrices: (128, 7, 512); chunk 6 row 0 holds row 768.
                # Issued mid-pipeline (ACT queue) so the 3.2MB of weights
                # doesn't compete with the first token streams for DMA.
                g = ins[f"G{which}_{s}"]
                gs = singles.tile([128, 7, D], f16, tag=f"G{which}_{s}")
                nc.scalar.dma_start(
                    out=gs[:, 0:6, :],
                    in_=g[0:F, :].rearrange("(k p) n -> p k n", p=128),
                )
                nc.scalar.dma_start(out=gs[0:1, 6, :], in_=g[F : F + 1, :])
                st[s][f"G{which}"] = gs

            def bcast_mid(ap2d, lo, hi, cnt):
                # [128, K] slice -> [128, cnt, K] with a stride-0 middle dim
                sl = ap2d[:, lo:hi]
                return bass.AP(
                    tensor=sl.tensor, offset=sl.offset, ap=[sl.ap[0], [0, cnt], sl.ap[1]]
                )

            def bcast_last(ap2d, lo, hi, cnt):
                # [128, K] slice -> [128, K, cnt] with a stride-0 last dim
                sl = ap2d[:, lo:hi]
                return bass.AP(
                    tensor=sl.tensor, offset=sl.offset, ap=[sl.ap[0], sl.ap[1], [0, cnt]]
                )

            # ---- streaming + finishing per branch -------------------------
            gmax = max(
                hi - lo for sf in (True, False) for lo, hi in _groups(NTB, sf)
            )
            for si, s in enumerate(("f", "b")):
                d = st[s]
                psK = psU_K.tile([8, D], f32)  # rows 0-3: U0(b), rows 4-7: U1(b)
                ps1 = psU_1.tile([8, KD + 2], f32)  # col KD holds P0/P1
                groups = _groups(NTB, small_first=(si == 0))
                xsrc = ins[f"x_{s}"].rearrange("(p n) d -> p n d", n=NTB)

                for gi, (lo, hi) in enumerate(groups):
                    g = hi - lo
                    first_g = si == 0 and gi == 0
                    last_g = si == 1 and gi == len(groups) - 1
                    chunks = (
                        [(a, min(a + 3, g)) for a in range(0, g, 3)]
                        if (first_g or last_g)
                        else [(0, g)]
                    )

                    x = xp.tile([128, gmax, FP], f16, tag="x")
                    for c0, c1 in chunks:
                        nc.sync.dma_start(
                            out=x[:, c0:c1, :], in_=xsrc[:, lo + c0 : lo + c1, :]
                        )

                    prodS = scr.tile([128, gmax, F], f16, tag="prodS")
                    junkS = scr.tile([128, F], f16, tag="junkS")
                    sS = small.tile([128, g], f32, tag="sS")
                    for c0, c1 in chunks:
                        # elementwise x*v products; DVE runs fp16 in 2x mode
                        nc.vector.tensor_mul(
                            prodS[:, c0:c1, 0:MS],
                            x[:, c0:c1, 0:MS],
                            bcast_mid(d["vb"], 0, MS, c1 - c0),
                        )
                        if ps:
                            nc.gpsimd.tensor_mul(
                                prodS[:, c0:c1, MS:F],
                                x[:, c0:c1, MS:F],
                                bcast_mid(d["vb"], MS, F, c1 - c0),
                            )
                    # per-token sums: DVE does n<rn in one segmented reduce,
                    # ACT accumulates full rows for n>=rn
                    rg = min(rn, g)
                    nc.vector.tensor_reduce(
                        out=sS[:, 0:rg],
                        in_=prodS[:, 0:rg, :],
                        axis=mybir.AxisListType.X,
                        op=mybir.AluOpType.add,
                    )
                    for n in range(rg, g):
                        nc.scalar.activation(
                            out=junkS,
                            in_=prodS[:, n, :],
                            func=mybir.ActivationFunctionType.Copy,
                            accum_out=sS[:, n : n + 1],
                        )
                    p_raw = small.tile([128, g], f32, tag="p_raw")
                    nc.scalar.activation(
                        out=p_raw, in_=sS, func=mybir.ActivationFunctionType.Exp
                    )

                    # pp[:, n, c]: p * m8 selects batch column + speaker group
                    pp = ppp.tile([128, g, 8], f16, tag="pp")
                    nc.vector.tensor_mul(
                        pp, d["m8"][:, lo:hi, :], bcast_last(p_raw, 0, g, 8)
                    )

                    for n in range(g):
                        first = gi == 0 and n == 0
                        last = gi == len(groups) - 1 and n == g - 1
                        nc.tensor.matmul(
                            psK, pp[:, n, :], x[:, n, 0:D], start=first, stop=last
                        )
                        nc.tensor.matmul(
                            ps1, pp[:, n, :], x[:, n, D:FP], start=first, stop=last
                        )

                    # stagger the weight loads into the pipeline
                    if s == "f" and gi == 1:
                        load_G("f", 0)
                        load_G("f", 1)
                    if s == "f" and gi == 3:
                        load_G("b", 0)
                        load_G("b", 1)

                # ---- finishing: out = (U0@G0 + U1@G1) / P ------------------
                uall = uallp.tile([8, F + 1], f16)
                nc.vector.tensor_copy(uall[:, 0:D], psK)
                nc.vector.tensor_copy(uall[:, D : F + 1], ps1[:, 0 : KD + 1])

                uallT = uallTp.tile([128, 7, 8], f16)
                for k in range(6):
                    trp = psTr.tile([128, 8], f16)
                    nc.tensor.transpose(trp, uall[:, k * 128 : (k + 1) * 128], identh[0:8, 0:8])
                    nc.vector.tensor_copy(uallT[:, k, :], trp)
                trp = psTr.tile([128, 8], f16)
                nc.tensor.transpose(trp[0:1, :], uall[:, F : F + 1], identh[0:8, 0:8])
                nc.vector.tensor_copy(uallT[0:1, 6, :], trp[0:1, :])

                po = psOut.tile([4, D], f32)
                for k in range(6):
                    nc.tensor.matmul(
                        po, uallT[:, k, 0:4], d["G0"][:, k, :], start=(k == 0), stop=False
                    )
                nc.tensor.matmul(po, uallT[0:1, 6, 0:4], d["G0"][0:1, 6, :], start=False, stop=False)
                for k in range(6):
                    nc.tensor.matmul(po, uallT[:, k, 4:8], d["G1"][:, k, :], start=False, stop=False)
                nc.tensor.matmul(po, uallT[0:1, 6, 4:8], d["G1"][0:1, 6, :], start=False, stop=True)

                psP4 = psPp.tile([4, 1], f32, tag="psP")
                nc.tensor.matmul(psP4, uallT[0:1, 6, 0:4], ones11h, start=True, stop=False)
                nc.tensor.matmul(psP4, uallT[0:1, 6, 4:8], ones11h, start=False, stop=True)

                rp = finp.tile([4, 1], f32, tag="rp")
                nc.vector.reciprocal(rp, psP4)
                osb = finp.tile([4, D], f32, tag="osb")
                nc.vector.tensor_scalar_mul(out=osb, in0=po, scalar1=rp)
                d["osb"] = osb

            # outputs issued at program end on the SP queue: by then all
            # stream dma_starts are already enqueued, so the dep waits here
            # cannot stall anything
            for s in ("f", "b"):
                nc.sync.dma_start(out=ins[f"out_{s}"][:, :], in_=st[s]["osb"])

    nc.compile()
    return nc


def _get_nc(NTB: int, ps: int, rn: int):
    key = (NTB, ps, rn)
    if key not in _BUILD_CACHE:
        _BUILD_CACHE[key] = _build(NTB, ps, rn)
    return _BUILD_CACHE[key]


def _pack(Kv, k1v, adj, sm, NTB):
    """Pack one core-branch: all 4 batch rows' adj=1 tokens concatenated,
    token j at partition j//NTB slot j%NTB; x gets [K | k1 | 1 | 0] rows."""
    f16 = np.float16
    CBc = 128 * NTB
    xc = np.zeros((CBc, FP), dtype=f16)
    m8 = np.zeros((128, NTB, 8), dtype=f16)
    pos = 0
    for b in range(Kv.shape[0]):
        idx = np.flatnonzero(adj[b])
        k = len(idx)
        xc[pos : pos + k, 0:D] = Kv[b, idx]
        xc[pos : pos + k, D:F] = k1v[b, idx]
        xc[pos : pos + k, F] = 1.0
        sl = sm[b, idx].astype(f16)
        j = np.arange(pos, pos + k)
        m8[j // NTB, j % NTB, b] = sl
        m8[j // NTB, j % NTB, 4 + b] = 1.0 - sl
        pos += k
    return xc, m8


def kernel(**inputs) -> tuple:
    global last_results
    from concourse.bass_utils import run_bass_kernel_spmd

    f32 = np.float32
    f16 = np.float16
    K = np.asarray(inputs["K"], dtype=f32)
    front_k1 = np.asarray(inputs["front_k1"], dtype=f32)
    back_K = np.asarray(inputs["back_K"], dtype=f32)
    back_k2 = np.asarray(inputs["back_k2"], dtype=f32)
    Wfk = np.asarray(inputs["Wfk"], dtype=f32)
    bfk = np.asarray(inputs["bfk"], dtype=f32)
    Wbk = np.asarray(inputs["Wbk"], dtype=f32)
    bbk = np.asarray(inputs["bbk"], dtype=f32)
    Wr0 = np.asarray(inputs["Wr0"], dtype=f32)
    Wr1 = np.asarray(inputs["Wr1"], dtype=f32)
    wf_den = np.asarray(inputs["wf_den"], dtype=f32)
    wb_den = np.asarray(inputs["wb_den"], dtype=f32)
    adj_f = np.asarray(inputs["front_sdj_den"], dtype=np.int32)
    sm_f = np.asarray(inputs["front_s_mask"], dtype=np.int32)
    adj_b = np.asarray(inputs["back_sdj_den"], dtype=np.int32)
    sm_b = np.asarray(inputs["back_s_mask"], dtype=np.int32)
    i = int(np.asarray(inputs["i"]))
    num_utter = int(np.asarray(inputs["num_utter"]))

    # host-folded weights
    v_f = (Wfk.astype(np.float64) @ wf_den[D:].astype(np.float64)).astype(f16)
    v_b = (Wbk.astype(np.float64) @ wb_den[D:].astype(np.float64)).astype(f16)
    A_f = np.vstack([Wfk, bfk[None, :]]).astype(np.float64)
    A_b = np.vstack([Wbk, bbk[None, :]]).astype(np.float64)
    G0_f = (A_f @ Wr0.astype(np.float64)).astype(f16)
    G1_f = (A_f @ Wr1.astype(np.float64)).astype(f16)
    G0_b = (A_b @ Wr0.astype(np.float64)).astype(f16)
    G1_b = (A_b @ Wr1.astype(np.float64)).astype(f16)

    # packed context length (adj=0 tokens contribute exactly 0)
    per_cb_f = adj_f.reshape(NCORES, BL, N).sum(axis=(1, 2))
    per_cb_b = adj_b.reshape(NCORES, BL, N).sum(axis=(1, 2))
    maxcnt = max(int(per_cb_f.max()), int(per_cb_b.max()), 1)
    NTB = min((BL * N) // 128, (maxcnt + 127) // 128)

    ps = int(os.environ.get("KERNEL_PS", "256"))
    rn = int(os.environ.get("KERNEL_RN", "3"))
    nc = _get_nc(NTB, ps, rn)

    in_maps = []
    for c in range(NCORES):
        sl = slice(c * BL, (c + 1) * BL)
        x_f, m8_f = _pack(K[sl], front_k1[sl], adj_f[sl], sm_f[sl], NTB)
        x_b, m8_b = _pack(back_K[sl], back_k2[sl], adj_b[sl], sm_b[sl], NTB)
        in_maps.append(
            {
                "x_f": x_f,
                "x_b": x_b,
                "m8_f": m8_f,
                "m8_b": m8_b,
                "v_f": v_f,
                "v_b": v_b,
                "G0_f": G0_f,
                "G1_f": G1_f,
                "G0_b": G0_b,
                "G1_b": G1_b,
            }
        )

    trace = os.environ.get("KERNEL_TRACE", "0") == "1"
    res = run_bass_kernel_spmd(nc, in_maps, core_ids=list(range(NCORES)), trace=trace)
    last_results = res

    front = np.concatenate([r["out_f"] for r in res.results], axis=0)
    back = np.concatenate([r["out_b"] for r in res.results], axis=0)
    if i == 0:
        front = np.zeros((B, D), dtype=f32)
    if i == num_utter - 1:
        back = np.zeros((B, D), dtype=f32)
    return (front, back)
